# revision 1
# baseline (speedup 1.0000x reference)
"""Trainium2 Bass kernel for nn_Consistent_loss_up_2 (scatter_memory).

Reference computation:
    bins = round(up*50+110) clipped to [0,255]; mask = up >= 0.0235
    scatter-max over i into up2left/up2right[k, 0, j, bin]:
        i > 128:  value (i-128)/60  -> up2right
        i <= 128: value (128-i)/60  -> up2left
    loss = mean(|up2right-right| masked) + mean(|up2left-left| masked)
    where masked = (d < 0.2) & (map != 0)

Key structure exploited:
  * only bins 111..160 (50 of 256) are reachable -> tables are [j, 100]
    (left at cols [0,50), right at [50,100))
  * scatter values are monotone in i, so scatter-max == overwrite-scatter
    in the right stream order (left: i descending, right: i ascending);
    gpsimd local_scatter is last-write-wins (verified on HW)
  * the left-stream reversal is folded into the PE transpose by using an
    anti-diagonal "identity" matrix
  * final output is a scalar: each core returns [128,2] partial sums

Sharding: data-parallel over batch B=128 across 8 cores (16 each).

Engine budget per core (cost model): DVE ~32us, ACT ~21us, Pool ~16us,
PE ~9us, DMA ~6MB.
"""

import numpy as np

from concourse import bacc, mybir, tile
from concourse.bass_utils import run_bass_kernel_spmd

B, H, W = 128, 256, 256
NCORES = 8
KPC = B // NCORES  # batches per core = 16
NBIN = 50          # reachable bins: 111..160
OFF = 1024.0       # table-value offset so empty bins auto-fail the d<0.2 test
R23 = 8388608.0    # 2^23: round-to-nearest-even trick
MASK_SUB = 4000.0  # pushes masked points' indices negative
TBLW = 100         # per-k table width: left at [0,50), right at [50,100)
REFW = TBLW * KPC  # staged width per j-tile = 1600

_cache = {}


def _build_bass():
    nc = bacc.Bacc("TRN2", target_bir_lowering=False)
    f32, i16 = mybir.dt.float32, mybir.dt.int16
    Alu = mybir.AluOpType
    Act = mybir.ActivationFunctionType

    up_in = nc.dram_tensor("up_in", [KPC * H, W], f32, kind="ExternalInput")
    refs_in = nc.dram_tensor("refs_in", [W, REFW], f32, kind="ExternalInput")
    vee_in = nc.dram_tensor("vee_in", [128, 256], i16, kind="ExternalInput")
    ro1_in = nc.dram_tensor("ro1_in", [128, 1], f32, kind="ExternalInput")
    off_in = nc.dram_tensor("off_in", [128, 1], f32, kind="ExternalInput")
    ident_in = nc.dram_tensor("ident_in", [128, 128], f32, kind="ExternalInput")
    antid_in = nc.dram_tensor("antid_in", [128, 128], f32, kind="ExternalInput")
    out = nc.dram_tensor("out", [128, 2], f32, kind="ExternalOutput")

    with tile.TileContext(nc) as tc:
        with (
            tc.tile_pool(name="const", bufs=1) as constp,
            tc.tile_pool(name="stage", bufs=1) as stagep,
            tc.tile_pool(name="work", bufs=3) as workp,
            tc.tile_pool(name="psum", bufs=4, space="PSUM") as psump,
            tc.tile_pool(name="loss", bufs=1) as lossp,
        ):
            vee = constp.tile([128, 256], i16)
            nc.sync.dma_start(vee[:], vee_in[:])
            ro1 = constp.tile([128, 1], f32)
            nc.sync.dma_start(ro1[:], ro1_in[:])
            offc = constp.tile([128, 1], f32)
            nc.sync.dma_start(offc[:], off_in[:])
            ident = constp.tile([128, 128], f32)
            nc.sync.dma_start(ident[:], ident_in[:])
            antid = constp.tile([128, 128], f32)
            nc.sync.dma_start(antid[:], antid_in[:])

            refs_sb = []
            tbl = []
            for jt in range(2):
                r = stagep.tile([128, REFW], f32, tag=f"refs{jt}")
                nc.scalar.dma_start(r[:], refs_in[jt * 128:(jt + 1) * 128, :])
                refs_sb.append(r)
                tbl.append(
                    stagep.tile([128, REFW], i16, tag=f"tbl{jt}", name=f"tbl{jt}")
                )

            for k in range(KPC):
                # one DMA per k: partition p <- rows (256k+p, 256k+128+p)
                ut = workp.tile([128, 2, W], f32, tag="ut")
                src = up_in[k * H:(k + 1) * H, :].rearrange(
                    "(h p) w -> p h w", h=2
                )
                nc.sync.dma_start(ut[:], src)
                utm = ut[:].rearrange("p h w -> p (h w)")

                # f = 50*u + 110 on ScalarE (both halves at once)
                fm = workp.tile([128, 2 * W], f32, tag="fm")
                nc.scalar.activation(
                    fm[:], utm, Act.Copy, bias=110.0, scale=50.0
                )
                # mask term on DVE (both halves at once)
                mk = workp.tile([128, 2 * W], f32, tag="mk")
                nc.vector.tensor_scalar(
                    mk[:], utm, 0.0235, MASK_SUB, op0=Alu.is_lt, op1=Alu.mult
                )
                # RNE rounding + per-half bin offset
                rbm = workp.tile([128, 2 * W], f32, tag="rbm")
                nc.vector.tensor_scalar(
                    rbm[:, 0:W], fm[:, 0:W], R23, R23 + 111.0,
                    op0=Alu.add, op1=Alu.subtract,
                )
                nc.vector.tensor_scalar(
                    rbm[:, W:2 * W], fm[:, W:2 * W], R23, ro1[:, :],
                    op0=Alu.add, op1=Alu.subtract,
                )
                ixm = workp.tile([128, 2 * W], f32, tag="ixm")
                nc.vector.tensor_tensor(
                    out=ixm[:], in0=rbm[:], in1=mk[:], op=Alu.subtract
                )

                for jt in range(2):
                    js = slice(jt * 128, (jt + 1) * 128)
                    ps = psump.tile([128, 256], f32, tag=f"ps{jt}", space="PSUM")
                    # anti-diagonal identity reverses columns: col n <-> i=127-n
                    nc.tensor.transpose(ps[:, 0:128], ixm[:, js], antid[:])
                    nc.tensor.transpose(
                        ps[:, 128:256], ixm[:, 256 + jt * 128:256 + (jt + 1) * 128],
                        ident[:],
                    )
                    st = workp.tile([128, 256], i16, tag=f"st{jt}")
                    nc.scalar.activation(st[:], ps[:], Act.Copy)

                    nc.gpsimd.local_scatter(
                        tbl[jt][:, k * TBLW:(k + 1) * TBLW],
                        vee[:],
                        st[:],
                        channels=128,
                        num_elems=TBLW,
                        num_idxs=256,
                    )

            for jt in range(2):
                e = lossp.tile([128, REFW], f32, tag="e")
                nc.vector.scalar_tensor_tensor(
                    e[:], refs_sb[jt][:], 60.0, tbl[jt][:],
                    op0=Alu.mult, op1=Alu.subtract,
                )
                a = lossp.tile([128, REFW], f32, tag="a")
                nc.scalar.activation(
                    a[:], e[:], Act.Abs, bias=offc[:, :], scale=1.0
                )
                cm = lossp.tile([128, REFW], f32, tag="cm")
                nc.vector.tensor_scalar(cm[:], a[:], 12.0, None, op0=Alu.is_lt)
                m = lossp.tile([128, REFW], f32, tag="m")
                nc.vector.tensor_tensor(
                    out=m[:], in0=a[:], in1=cm[:], op=Alu.mult
                )
                junk = lossp.tile([128, REFW], f32, tag="junk")
                part = lossp.tile([128, 1], f32, tag=f"part{jt}")
                nc.scalar.activation(
                    junk[:], m[:], Act.Copy, accum_out=part[:]
                )
                nc.scalar.dma_start(out[:, jt:jt + 1], part[:])

    nc.compile()
    return nc


def _host_constants():
    # scatter data stream values:
    #   pos n in [0,128): i = 127-n (left)  -> value (128-i)+OFF = n+1+OFF
    #   pos n in [128,256): i = n (right)   -> value (i-128)+OFF = n-128+OFF
    n = np.arange(256)
    vee = np.where(n < 128, n + 1, n - 128).astype(np.int16) + np.int16(OFF)
    vee = np.ascontiguousarray(np.broadcast_to(vee, (128, 256)))

    # per-partition subtrahend for tile1 (i = 128+p):
    #   p=0 is i=128 -> always skip; p>=1 -> right table at +50
    ro1 = np.full((128, 1), R23 + 61.0, np.float32)
    ro1[0, 0] = R23 + 311.0

    ident = np.eye(128, dtype=np.float32)
    antid = np.ascontiguousarray(ident[::-1, :])
    return vee, ro1, ident, antid


def _prep_refs(left, right):
    """[256, REFW] per core: row j (jt*128+p), col k*100+[0:50)=left slice,
    k*100+50+[0:50)=right slice."""
    lft = left[:, 0, :, 111:161]   # [B, W, 50]
    rgt = right[:, 0, :, 111:161]
    refs = np.zeros((NCORES, 2, 128, KPC, TBLW), np.float32)
    lv = lft.reshape(NCORES, KPC, 2, 128, NBIN).transpose(0, 2, 3, 1, 4)
    rv = rgt.reshape(NCORES, KPC, 2, 128, NBIN).transpose(0, 2, 3, 1, 4)
    refs[..., 0:NBIN] = lv
    refs[..., NBIN:2 * NBIN] = rv
    return refs.reshape(NCORES, W, REFW)


def make_in_maps(up, left, right):
    up = np.asarray(up, np.float32)
    left = np.asarray(left, np.float32)
    right = np.asarray(right, np.float32)
    vee, ro1, ident, antid = _host_constants()
    refs = _prep_refs(left, right)
    in_maps = []
    for c in range(NCORES):
        upc = np.ascontiguousarray(
            up[c * KPC:(c + 1) * KPC, 0].reshape(KPC * H, W)
        )
        in_maps.append({
            "up_in": upc,
            "refs_in": np.ascontiguousarray(refs[c]),
            "vee_in": vee,
            "ro1_in": ro1,
            "off_in": np.full((128, 1), OFF, np.float32),
            "ident_in": ident,
            "antid_in": antid,
        })
    return in_maps


def get_nc():
    if "nc" not in _cache:
        _cache["nc"] = _build_bass()
    return _cache["nc"]


def reduce_results(results):
    total = 0.0
    for r in results:
        total += float(r["out"].astype(np.float64).sum())
    return np.float32(total / (60.0 * B * W * W))


def kernel(up, left, right):
    nc = get_nc()
    in_maps = make_in_maps(up, left, right)
    res = run_bass_kernel_spmd(nc, in_maps, core_ids=list(range(NCORES)))
    return reduce_results(res.results)



# revision 6
# speedup vs baseline: 1.0966x; 1.0966x over previous
"""Trainium2 Bass kernel for nn_Consistent_loss_up_2 (scatter_memory).

Reference computation:
    bins = round(up*50+110) clipped to [0,255]; mask = up >= 0.0235
    scatter-max over i into up2left/up2right[k, 0, j, bin]:
        i > 128:  value (i-128)/60  -> up2right
        i <= 128: value (128-i)/60  -> up2left
    loss = mean(|up2right-right| masked) + mean(|up2left-left| masked)
    where masked = (d < 0.2) & (map != 0)

Key structure exploited:
  * only bins 111..160 (50 of 256) are reachable -> per-(k,j) tables are
    50 wide; left/right/jt packed into 200 cols per k
  * scatter values are monotone in i, so scatter-max == overwrite-scatter
    in the right stream order (left: i descending, right: i ascending);
    gpsimd local_scatter is last-write-wins (verified on HW)
  * bin rounding is done by the f32->fp16 dtype conversion in the ACT
    g-pass (values kept in [1044,1094] where fp16 ulp=1 so RNE rounds to
    exact ints, matching jnp.round) -- no 2^23 trick, no extra DVE pass
  * per-(jt,h) table column offsets {0,50,100,150} are added pre-transpose
    via a constant cofs tile folded into the mask subtrahend
  * scatter calls are merged 2 batches x 4 blocks -> 8 calls of 1024 idxs
    (call overhead ~200ns, marginal ~4.1ns/idx-col)
  * loss phase is chunked (4 chunks, separate table tiles) so it overlaps
    the scatter phase instead of serializing at the end

Sharding: data-parallel over batch B=128 across 8 cores (16 each);
each core returns [128,4] partial sums, host reduces.
"""

import numpy as np

from concourse import bacc, mybir, tile
from concourse.bass_utils import run_bass_kernel_spmd

B, H, W = 128, 256, 256
NCORES = 8
KPC = B // NCORES   # batches per core = 16
NBIN = 50           # reachable bins: 111..160
OFF = 1024.0        # table-value offset so empty bins auto-fail d<0.2
MASK_SUB = 4000.0   # pushes masked points' indices negative
KTBL = 200          # per-k table: [jt0L, jt0R, jt1L, jt1R] x 50 bins
NPAIR = KPC // 2    # scatter pairs = 8
CHUNKS = 4          # loss chunks
KPCH = KPC // CHUNKS        # 4 k per chunk
CHW = KPCH * KTBL           # 800 cols per chunk tile

_cache = {}


def _build_bass():
    nc = bacc.Bacc("TRN2", target_bir_lowering=False)
    f32, i16, f16 = mybir.dt.float32, mybir.dt.int16, mybir.dt.float16
    Alu = mybir.AluOpType
    Act = mybir.ActivationFunctionType

    up_in = nc.dram_tensor("up_in", [KPC * H, W], f32, kind="ExternalInput")
    refs_in = nc.dram_tensor("refs_in", [CHUNKS * 128, CHW], f32,
                             kind="ExternalInput")
    vee_in = nc.dram_tensor("vee_in", [128, 1024], i16, kind="ExternalInput")
    cofs_in = nc.dram_tensor("cofs_in", [128, 512], f16, kind="ExternalInput")
    ident_in = nc.dram_tensor("ident_in", [128, 128], f16, kind="ExternalInput")
    antid_in = nc.dram_tensor("antid_in", [128, 128], f16, kind="ExternalInput")
    out = nc.dram_tensor("out", [128, CHUNKS], f32, kind="ExternalOutput")

    with tile.TileContext(nc) as tc:
        with (
            tc.tile_pool(name="const", bufs=1) as constp,
            tc.tile_pool(name="tblp", bufs=1) as tblp,
            tc.tile_pool(name="refp", bufs=1) as refp,
            tc.tile_pool(name="work", bufs=3) as workp,
            tc.tile_pool(name="stp", bufs=2) as stp,
            tc.tile_pool(name="psum", bufs=3, space="PSUM") as psump,
            tc.tile_pool(name="loss", bufs=2) as lossp,
        ):
            # prefetch the first up batches before anything else
            uts = {}
            for k in range(2):
                uts[k] = workp.tile([128, 2, W], f32, tag="ut", name=f"ut{k}")
                src = up_in[k * H:(k + 1) * H, :].rearrange(
                    "(h p) w -> p h w", h=2
                )
                nc.sync.dma_start(uts[k][:], src)

            vee = constp.tile([128, 1024], i16)
            nc.sync.dma_start(vee[:], vee_in[:])
            cofs = constp.tile([128, 512], f16)
            nc.sync.dma_start(cofs[:], cofs_in[:])
            ident = constp.tile([128, 128], f16)
            nc.sync.dma_start(ident[:], ident_in[:])
            antid = constp.tile([128, 128], f16)
            nc.sync.dma_start(antid[:], antid_in[:])

            # refs chunks on the scalar-queue DMA so they don't block ut
            refs_sb = []
            for c in range(CHUNKS):
                r = refp.tile([128, CHW], f32, tag=f"refs{c}")
                nc.scalar.dma_start(r[:], refs_in[c * 128:(c + 1) * 128, :])
                refs_sb.append(r)

            tbl = [
                tblp.tile([128, CHW], i16, tag=f"tbl{c}", name=f"tbl{c}")
                for c in range(CHUNKS)
            ]
            parts = [
                lossp.tile([128, 1], f32, tag=f"part{c}", name=f"part{c}")
                for c in range(CHUNKS)
            ]

            sts = {}
            for k in range(KPC):
                if k not in uts:
                    uts[k] = workp.tile([128, 2, W], f32, tag="ut", name=f"ut{k}")
                    src = up_in[k * H:(k + 1) * H, :].rearrange(
                        "(h p) w -> p h w", h=2
                    )
                    nc.sync.dma_start(uts[k][:], src)
                ut = uts.pop(k)
                utm = ut[:].rearrange("p h w -> p (h w)")

                # g = bf16(50*u + 130): rounding via dtype conversion
                g = workp.tile([128, 512], f16, tag="g")
                nc.scalar.activation(g[:], utm, Act.Copy, bias=1044.0, scale=50.0)
                # mkx = (u < 0.0235)*1000 - cofs  (cofs = block col offsets)
                mk = workp.tile([128, 512], f16, tag="mk")
                nc.vector.tensor_scalar(
                    mk[:], utm, 0.0235, MASK_SUB, op0=Alu.is_lt, op1=Alu.mult
                )
                mkx = workp.tile([128, 512], f16, tag="mkx")
                nc.vector.tensor_tensor(
                    out=mkx[:], in0=mk[:], in1=cofs[:], op=Alu.subtract
                )
                # ix = g - mkx = bin+20 + cofs (valid) | very negative (masked)
                ix = workp.tile([128, 512], f16, tag="ix")
                nc.vector.tensor_tensor(
                    out=ix[:], in0=g[:], in1=mkx[:], op=Alu.subtract
                )

                # transpose to partition=j: blocks [jt0h0, jt0h1, jt1h0, jt1h1]
                ps = psump.tile([128, 512], f16, tag="ps", space="PSUM")
                nc.tensor.transpose(ps[:, 0:128], ix[:, 0:128], antid[:])
                nc.tensor.transpose(ps[:, 128:256], ix[:, 256:384], ident[:])
                nc.tensor.transpose(ps[:, 256:384], ix[:, 128:256], antid[:])
                nc.tensor.transpose(ps[:, 384:512], ix[:, 384:512], ident[:])

                pair = k // 2
                if k % 2 == 0:
                    sts[pair] = stp.tile([128, 1024], i16, tag="st", name=f"st{pair}")
                # idx = ps - 131 (+200 for the odd k of the pair)
                nc.scalar.activation(
                    sts[pair][:, (k % 2) * 512:(k % 2) * 512 + 512], ps[:],
                    Act.Copy, bias=-1045.0 + 200.0 * (k % 2), scale=1.0,
                )

                if k % 2 == 1:
                    st = sts.pop(pair)
                    c = pair // 2
                    half = pair % 2
                    nc.gpsimd.local_scatter(
                        tbl[c][:, half * 2 * KTBL:(half + 1) * 2 * KTBL],
                        vee[:],
                        st[:],
                        channels=128,
                        num_elems=2 * KTBL,
                        num_idxs=1024,
                    )
                    if half == 1:
                        # chunk c complete: loss piece, overlapped with next k's
                        e2 = lossp.tile([128, CHW], f32, tag="e2")
                        nc.vector.tensor_tensor(
                            out=e2[:], in0=refs_sb[c][:], in1=tbl[c][:],
                            op=Alu.subtract,
                        )
                        a = lossp.tile([128, CHW], f32, tag="a")
                        nc.scalar.activation(a[:], e2[:], Act.Abs)
                        cm = lossp.tile([128, CHW], f32, tag="cm")
                        nc.vector.tensor_scalar(
                            cm[:], a[:], 12.0, None, op0=Alu.is_lt
                        )
                        junk = lossp.tile([128, CHW], f32, tag="junk")
                        nc.vector.scalar_tensor_tensor(
                            junk[:], a[:], 1.0, cm[:],
                            op0=Alu.mult, op1=Alu.mult,
                            accum_out=parts[c][:],
                        )
                        nc.scalar.dma_start(out[:, c:c + 1], parts[c][:])

    nc.compile()
    return nc


def _host_constants():
    # scatter data stream values per 256-block: [left: n+1 | right: n-128],
    # +OFF; garbage 25000 at the right-stream head (i==128, value 0 in the
    # reference -> must never produce a live table entry by itself)
    n = np.arange(256)
    blk = np.where(n < 128, n + 1, n - 128).astype(np.int64) + int(OFF)
    blk[128] = 25000
    vee = np.tile(blk, 4).astype(np.int16)
    vee = np.ascontiguousarray(np.broadcast_to(vee, (128, 1024)))

    # cofs[col]: block col offset {0,50,100,150} for [jt0L, jt0R, jt1L, jt1R]
    # pre-transpose layout: col = h*256 + j ; block = (jt, h)
    col = np.arange(512)
    h = col // 256
    j = col % 256
    jt = (j // 128).astype(np.int64)
    cofs = (100 * jt + 50 * h).astype(np.float32)
    cofs = np.ascontiguousarray(
        np.broadcast_to(cofs.astype(np.float16), (128, 512))
    )

    ident = np.eye(128, dtype=np.float32).astype(np.float16)
    antid = np.ascontiguousarray(ident[::-1, :])
    return vee, cofs, ident, antid


def _prep_refs(left, right):
    """[CHUNKS*128, CHW] per core; table col (within core) =
    k*200 + block*50 + (bin-111), block in [jt0L, jt0R, jt1L, jt1R],
    channel = j mod 128; values pre-scaled: 60*ref + OFF."""
    lft = left[:, 0, :, 111:161]    # [B, W, 50]
    rgt = right[:, 0, :, 111:161]
    # refs[core, k, block, channel, bin]
    refs = np.empty((NCORES, KPC, 4, 128, NBIN), np.float32)
    for core in range(NCORES):
        for k in range(KPC):
            kg = core * KPC + k
            refs[core, k, 0] = lft[kg, 0:128, :]
            refs[core, k, 1] = rgt[kg, 0:128, :]
            refs[core, k, 2] = lft[kg, 128:256, :]
            refs[core, k, 3] = rgt[kg, 128:256, :]
    refs = refs * 60.0 + np.float32(OFF)
    # -> [core, chunk, channel, (k_in_chunk, block, bin)]
    refs = refs.reshape(NCORES, CHUNKS, KPCH, 4, 128, NBIN)
    refs = refs.transpose(0, 1, 4, 2, 3, 5)
    return np.ascontiguousarray(
        refs.reshape(NCORES, CHUNKS * 128, CHW).astype(np.float32)
    )


def make_in_maps(up, left, right):
    up = np.asarray(up, np.float32)
    left = np.asarray(left, np.float32)
    right = np.asarray(right, np.float32)
    vee, cofs, ident, antid = _host_constants()
    refs = _prep_refs(left, right)
    in_maps = []
    for c in range(NCORES):
        upc = np.ascontiguousarray(
            up[c * KPC:(c + 1) * KPC, 0].reshape(KPC * H, W)
        )
        in_maps.append({
            "up_in": upc,
            "refs_in": refs[c],
            "vee_in": vee,
            "cofs_in": cofs,
            "ident_in": ident,
            "antid_in": antid,
        })
    return in_maps


def get_nc():
    if "nc" not in _cache:
        _cache["nc"] = _build_bass()
    return _cache["nc"]


def reduce_results(results):
    total = 0.0
    for r in results:
        total += float(np.asarray(r["out"]).astype(np.float64).sum())
    return np.float32(total / (60.0 * B * W * W))


def kernel(up, left, right):
    nc = get_nc()
    in_maps = make_in_maps(up, left, right)
    res = run_bass_kernel_spmd(nc, in_maps, core_ids=list(range(NCORES)))
    return reduce_results(res.results)


# revision 7
# speedup vs baseline: 1.1350x; 1.0351x over previous
"""Trainium2 Bass kernel for nn_Consistent_loss_up_2 (scatter_memory).

Reference computation:
    bins = round(up*50+110) clipped to [0,255]; mask = up >= 0.0235
    scatter-max over i into up2left/up2right[k, 0, j, bin]:
        i > 128:  value (i-128)/60  -> up2right
        i <= 128: value (128-i)/60  -> up2left
    loss = mean(|up2right-right| masked) + mean(|up2left-left| masked)
    where masked = (d < 0.2) & (map != 0)

Key structure exploited:
  * only bins 111..160 (50 of 256) are reachable -> per-(k,j) tables are
    50 wide; left/right/jt/k-parity packed into 400 cols per k-pair
  * scatter values are monotone in i, so scatter-max == overwrite-scatter
    in the right stream order (left: i descending, right: i ascending);
    gpsimd local_scatter is last-write-wins (verified on HW)
  * bin rounding is done by the f32->fp16 dtype conversion in the ACT
    g-pass (values kept in [1044,1094] where fp16 ulp=1 so RNE rounds to
    exact ints, matching jnp.round) -- no 2^23 trick, no extra DVE pass
  * per-(k%2,jt,h) table column offsets {0,...,350} are added pre-transpose
    via a constant cofs tile folded into the mask subtrahend; all values
    stay < 2048 so fp16 arithmetic on them is exact
  * two batches are processed per iteration with 1024-wide ops: ACT calls
    cost ~700ns nearly independent of width, so fewer/wider wins
  * scatter calls are merged into 8 calls x 1024 idxs (call overhead
    ~200ns, marginal ~4.1ns/idx-col)
  * loss phase is chunked (4 chunks, separate table tiles) so it overlaps
    the scatter phase instead of serializing at the end

Sharding: data-parallel over batch B=128 across 8 cores (16 each);
each core returns [128,4] partial sums, host reduces.
"""

import numpy as np

from concourse import bacc, mybir, tile
from concourse.bass_utils import run_bass_kernel_spmd

B, H, W = 128, 256, 256
NCORES = 8
KPC = B // NCORES   # batches per core = 16
NBIN = 50           # reachable bins: 111..160
OFF = 1024.0        # table-value offset so empty bins auto-fail d<0.2
MASK_SUB = 4000.0   # pushes masked points' indices negative
KTBL = 200          # per-k table: [jt0L, jt0R, jt1L, jt1R] x 50 bins
NPAIR = KPC // 2    # 8 k-pairs per core
CHUNKS = 4          # loss chunks
CHW = 2 * 2 * KTBL  # 800 cols per chunk tile (2 pairs)

_cache = {}


def _build_bass():
    nc = bacc.Bacc("TRN2", target_bir_lowering=False)
    f32, i16, f16 = mybir.dt.float32, mybir.dt.int16, mybir.dt.float16
    Alu = mybir.AluOpType
    Act = mybir.ActivationFunctionType

    up_in = nc.dram_tensor("up_in", [KPC * H, W], f32, kind="ExternalInput")
    refs_in = nc.dram_tensor("refs_in", [CHUNKS * 128, CHW], f32,
                             kind="ExternalInput")
    vee_in = nc.dram_tensor("vee_in", [128, 1024], i16, kind="ExternalInput")
    cofs_in = nc.dram_tensor("cofs_in", [128, 1024], f16, kind="ExternalInput")
    ident_in = nc.dram_tensor("ident_in", [128, 128], f16, kind="ExternalInput")
    antid_in = nc.dram_tensor("antid_in", [128, 128], f16, kind="ExternalInput")
    out = nc.dram_tensor("out", [128, CHUNKS], f32, kind="ExternalOutput")

    with tile.TileContext(nc) as tc:
        with (
            tc.tile_pool(name="const", bufs=1) as constp,
            tc.tile_pool(name="tblp", bufs=1) as tblp,
            tc.tile_pool(name="refp", bufs=1) as refp,
            tc.tile_pool(name="work", bufs=3) as workp,
            tc.tile_pool(name="psum", bufs=3, space="PSUM") as psump,
            tc.tile_pool(name="loss", bufs=2) as lossp,
        ):
            # prefetch the first up pairs before anything else
            uts = {}
            for p in range(2):
                uts[p] = workp.tile([128, 4, W], f32, tag="ut", name=f"ut{p}")
                src = up_in[p * 2 * H:(p + 1) * 2 * H, :].rearrange(
                    "(q p) w -> p q w", q=4
                )
                nc.sync.dma_start(uts[p][:], src)

            vee = constp.tile([128, 1024], i16)
            nc.scalar.dma_start(vee[:], vee_in[:])
            cofs = constp.tile([128, 1024], f16)
            nc.scalar.dma_start(cofs[:], cofs_in[:])
            ident = constp.tile([128, 128], f16)
            nc.scalar.dma_start(ident[:], ident_in[:])
            antid = constp.tile([128, 128], f16)
            nc.scalar.dma_start(antid[:], antid_in[:])

            # refs chunks on the scalar-queue DMA so they don't block ut
            refs_sb = []
            for c in range(CHUNKS):
                r = refp.tile([128, CHW], f32, tag=f"refs{c}")
                nc.scalar.dma_start(r[:], refs_in[c * 128:(c + 1) * 128, :])
                refs_sb.append(r)

            tbl = [
                tblp.tile([128, CHW], i16, tag=f"tbl{c}", name=f"tbl{c}")
                for c in range(CHUNKS)
            ]
            parts = [
                lossp.tile([128, 1], f32, tag=f"part{c}", name=f"part{c}")
                for c in range(CHUNKS)
            ]

            for p in range(NPAIR):
                if p not in uts:
                    uts[p] = workp.tile([128, 4, W], f32, tag="ut",
                                        name=f"ut{p}")
                    src = up_in[p * 2 * H:(p + 1) * 2 * H, :].rearrange(
                        "(q p) w -> p q w", q=4
                    )
                    nc.sync.dma_start(uts[p][:], src)
                ut = uts.pop(p)
                utm = ut[:].rearrange("p q w -> p (q w)")

                # g = fp16(50*u + 1044): rounding via dtype conversion
                g = workp.tile([128, 1024], f16, tag="g")
                nc.scalar.activation(g[:], utm, Act.Copy, bias=1044.0,
                                     scale=50.0)
                # mkx = (u < 0.0235)*4000 - cofs
                mk = workp.tile([128, 1024], f16, tag="mk")
                nc.vector.tensor_scalar(
                    mk[:], utm, 0.0235, MASK_SUB, op0=Alu.is_lt, op1=Alu.mult
                )
                mkx = workp.tile([128, 1024], f16, tag="mkx")
                nc.vector.tensor_tensor(
                    out=mkx[:], in0=mk[:], in1=cofs[:], op=Alu.subtract
                )
                # ix = g - mkx: bin+1044+cofs (valid) | very negative (masked)
                ix = workp.tile([128, 1024], f16, tag="ix")
                nc.vector.tensor_tensor(
                    out=ix[:], in0=g[:], in1=mkx[:], op=Alu.subtract
                )

                # transpose to partition=j; 8 blocks (k-parity, jt, h)
                ps = psump.tile([128, 1024], f16, tag="ps", space="PSUM")
                for kk in range(2):
                    o = kk * 512
                    nc.tensor.transpose(ps[:, o:o + 128],
                                        ix[:, o:o + 128], antid[:])
                    nc.tensor.transpose(ps[:, o + 128:o + 256],
                                        ix[:, o + 256:o + 384], ident[:])
                    nc.tensor.transpose(ps[:, o + 256:o + 384],
                                        ix[:, o + 128:o + 256], antid[:])
                    nc.tensor.transpose(ps[:, o + 384:o + 512],
                                        ix[:, o + 384:o + 512], ident[:])

                # idx = ps - 1045 in [0,400) valid, negative = skip
                st = workp.tile([128, 1024], i16, tag="st")
                nc.scalar.activation(st[:], ps[:], Act.Copy, bias=-1045.0,
                                     scale=1.0)

                c = p // 2
                half = p % 2
                nc.gpsimd.local_scatter(
                    tbl[c][:, half * 2 * KTBL:(half + 1) * 2 * KTBL],
                    vee[:],
                    st[:],
                    channels=128,
                    num_elems=2 * KTBL,
                    num_idxs=1024,
                )
                if half == 1:
                    # chunk c complete: loss piece, overlapped with next pairs
                    e2 = lossp.tile([128, CHW], f32, tag="e2")
                    nc.vector.tensor_tensor(
                        out=e2[:], in0=refs_sb[c][:], in1=tbl[c][:],
                        op=Alu.subtract,
                    )
                    a = lossp.tile([128, CHW], f32, tag="a")
                    nc.scalar.activation(a[:], e2[:], Act.Abs)
                    cm = lossp.tile([128, CHW], f32, tag="cm")
                    nc.vector.tensor_scalar(
                        cm[:], a[:], 12.0, None, op0=Alu.is_lt
                    )
                    junk = lossp.tile([128, CHW], f32, tag="junk")
                    nc.vector.scalar_tensor_tensor(
                        junk[:], a[:], 1.0, cm[:],
                        op0=Alu.mult, op1=Alu.mult,
                        accum_out=parts[c][:],
                    )
                    nc.scalar.dma_start(out[:, c:c + 1], parts[c][:])

    nc.compile()
    return nc


def _host_constants():
    # scatter data stream values per 256-block: [left: n+1 | right: n-128],
    # +OFF; garbage 25000 at the right-stream head (i==128, value 0 in the
    # reference -> must never produce a live table entry by itself)
    n = np.arange(256)
    blk = np.where(n < 128, n + 1, n - 128).astype(np.int64) + int(OFF)
    blk[128] = 25000
    vee = np.tile(blk, 4).astype(np.int16)
    vee = np.ascontiguousarray(np.broadcast_to(vee, (128, 1024)))

    # cofs[col]: block col offset {0,50,100,150} + 200*(k%2);
    # pre-transpose layout: col = (k%2)*512 + h*256 + j
    col = np.arange(1024)
    kk = col // 512
    h = (col % 512) // 256
    j = col % 256
    jt = (j // 128).astype(np.int64)
    cofs = (200 * kk + 100 * jt + 50 * h).astype(np.float32)
    cofs = np.ascontiguousarray(
        np.broadcast_to(cofs.astype(np.float16), (128, 1024))
    )

    ident = np.eye(128, dtype=np.float32).astype(np.float16)
    antid = np.ascontiguousarray(ident[::-1, :])
    return vee, cofs, ident, antid


def _prep_refs(left, right):
    """[CHUNKS*128, CHW] per core; table col (within core) =
    k*200 + block*50 + (bin-111), block in [jt0L, jt0R, jt1L, jt1R],
    channel = j mod 128; values pre-scaled: 60*ref + OFF."""
    lft = left[:, 0, :, 111:161]    # [B, W, 50]
    rgt = right[:, 0, :, 111:161]
    refs = np.empty((NCORES, KPC, 4, 128, NBIN), np.float32)
    for core in range(NCORES):
        for k in range(KPC):
            kg = core * KPC + k
            refs[core, k, 0] = lft[kg, 0:128, :]
            refs[core, k, 1] = rgt[kg, 0:128, :]
            refs[core, k, 2] = lft[kg, 128:256, :]
            refs[core, k, 3] = rgt[kg, 128:256, :]
    refs = refs * 60.0 + np.float32(OFF)
    # -> [core, chunk, channel, (k_in_chunk, block, bin)]
    refs = refs.reshape(NCORES, CHUNKS, KPC // CHUNKS, 4, 128, NBIN)
    refs = refs.transpose(0, 1, 4, 2, 3, 5)
    return np.ascontiguousarray(
        refs.reshape(NCORES, CHUNKS * 128, CHW).astype(np.float32)
    )


def make_in_maps(up, left, right):
    up = np.asarray(up, np.float32)
    left = np.asarray(left, np.float32)
    right = np.asarray(right, np.float32)
    vee, cofs, ident, antid = _host_constants()
    refs = _prep_refs(left, right)
    in_maps = []
    for c in range(NCORES):
        upc = np.ascontiguousarray(
            up[c * KPC:(c + 1) * KPC, 0].reshape(KPC * H, W)
        )
        in_maps.append({
            "up_in": upc,
            "refs_in": refs[c],
            "vee_in": vee,
            "cofs_in": cofs,
            "ident_in": ident,
            "antid_in": antid,
        })
    return in_maps


def get_nc():
    if "nc" not in _cache:
        _cache["nc"] = _build_bass()
    return _cache["nc"]


def reduce_results(results):
    total = 0.0
    for r in results:
        total += float(np.asarray(r["out"]).astype(np.float64).sum())
    return np.float32(total / (60.0 * B * W * W))


def kernel(up, left, right):
    nc = get_nc()
    in_maps = make_in_maps(up, left, right)
    res = run_bass_kernel_spmd(nc, in_maps, core_ids=list(range(NCORES)))
    return reduce_results(res.results)


# revision 8
# speedup vs baseline: 1.1776x; 1.0375x over previous
"""Trainium2 Bass kernel for nn_Consistent_loss_up_2 (scatter_memory).

Reference computation:
    bins = round(up*50+110) clipped to [0,255]; mask = up >= 0.0235
    scatter-max over i into up2left/up2right[k, 0, j, bin]:
        i > 128:  value (i-128)/60  -> up2right
        i <= 128: value (128-i)/60  -> up2left
    loss = mean(|up2right-right| masked) + mean(|up2left-left| masked)
    where masked = (d < 0.2) & (map != 0)

Key structure exploited:
  * only bins 111..160 (50 of 256) are reachable -> per-(k,j) tables are
    50 wide; left/right/jt/k-parity packed into 400 cols per k-pair
  * scatter values are monotone in i, so scatter-max == overwrite-scatter
    in the right stream order (left: i descending, right: i ascending);
    gpsimd local_scatter is last-write-wins (verified on HW)
  * bin rounding is done by the f32->fp16 dtype conversion in the ACT
    g-pass (values kept in [1044,1094] where fp16 ulp=1 so RNE rounds to
    exact ints, matching jnp.round) -- no 2^23 trick, no extra DVE pass
  * per-(k%2,jt,h) table column offsets {0,...,350} are added pre-transpose
    via a constant cofs tile folded into the mask subtrahend; all values
    stay < 2048 so fp16 arithmetic on them is exact
  * two batches are processed per iteration with 1024-wide ops: ACT calls
    cost ~700ns nearly independent of width, so fewer/wider wins
  * scatter calls are merged into 8 calls x 1024 idxs (call overhead
    ~200ns, marginal ~4.1ns/idx-col)
  * loss phase is chunked per pair (8 chunks, separate table tiles) and
    emitted 3 pairs late so it overlaps the scatter phase without ever
    stalling the in-order DVE queue; g for pair p+1 is emitted before
    st_p so the in-order ACT queue never serializes the pipeline

Sharding: data-parallel over batch B=128 across 8 cores (16 each);
each core returns [128,4] partial sums, host reduces.
"""

import numpy as np

from concourse import bacc, mybir, tile
from concourse.bass_utils import run_bass_kernel_spmd

B, H, W = 128, 256, 256
NCORES = 8
KPC = B // NCORES   # batches per core = 16
NBIN = 50           # reachable bins: 111..160
OFF = 1024.0        # table-value offset so empty bins auto-fail d<0.2
MASK_SUB = 4000.0   # pushes masked points' indices negative
KTBL = 200          # per-k table: [jt0L, jt0R, jt1L, jt1R] x 50 bins
NPAIR = KPC // 2    # 8 k-pairs per core
CHW = 2 * KTBL      # 400 cols per pair table tile

_cache = {}


def _build_bass():
    nc = bacc.Bacc("TRN2", target_bir_lowering=False)
    f32, i16, f16 = mybir.dt.float32, mybir.dt.int16, mybir.dt.float16
    Alu = mybir.AluOpType
    Act = mybir.ActivationFunctionType

    up_in = nc.dram_tensor("up_in", [KPC * H, W], f32, kind="ExternalInput")
    refs_in = nc.dram_tensor("refs_in", [NPAIR * 128, CHW], f32,
                             kind="ExternalInput")
    vee_in = nc.dram_tensor("vee_in", [128, 1024], i16, kind="ExternalInput")
    cofs_in = nc.dram_tensor("cofs_in", [128, 1024], f16, kind="ExternalInput")
    ident_in = nc.dram_tensor("ident_in", [128, 128], f16, kind="ExternalInput")
    antid_in = nc.dram_tensor("antid_in", [128, 128], f16, kind="ExternalInput")
    out = nc.dram_tensor("out", [128, NPAIR], f32, kind="ExternalOutput")

    with tile.TileContext(nc) as tc:
        with (
            tc.tile_pool(name="const", bufs=1) as constp,
            tc.tile_pool(name="tblp", bufs=1) as tblp,
            tc.tile_pool(name="refp", bufs=1) as refp,
            tc.tile_pool(name="work", bufs=4) as workp,
            tc.tile_pool(name="psum", bufs=4, space="PSUM") as psump,
            tc.tile_pool(name="loss", bufs=2) as lossp,
        ):
            # prefetch the first up pairs before anything else (sync queue)
            uts = {}

            def fetch_ut(p):
                uts[p] = workp.tile([128, 4, W], f32, tag="ut", name=f"ut{p}")
                src = up_in[p * 2 * H:(p + 1) * 2 * H, :].rearrange(
                    "(q p) w -> p q w", q=4
                )
                nc.sync.dma_start(uts[p][:], src)

            for p in range(3):
                fetch_ut(p)

            # small consts + vee/cofs on the gpsimd queue (idle until the
            # first scatter), perm matrices on sync (needed by PE early)
            vee = constp.tile([128, 1024], i16)
            nc.gpsimd.dma_start(vee[:], vee_in[:])
            cofs = constp.tile([128, 1024], f16)
            nc.gpsimd.dma_start(cofs[:], cofs_in[:])
            ident = constp.tile([128, 128], f16)
            nc.sync.dma_start(ident[:], ident_in[:])
            antid = constp.tile([128, 128], f16)
            nc.sync.dma_start(antid[:], antid_in[:])

            refs_sb = [None] * NPAIR

            def fetch_refs(p):
                r = refp.tile([128, CHW], f32, tag=f"refs{p}",
                              name=f"refs{p}")
                nc.sync.dma_start(r[:], refs_in[p * 128:(p + 1) * 128, :])
                refs_sb[p] = r

            for p in range(2):
                fetch_refs(p)

            tbl = [
                tblp.tile([128, CHW], i16, tag=f"tbl{p}", name=f"tbl{p}")
                for p in range(NPAIR)
            ]
            parts = [
                lossp.tile([128, 1], f32, tag=f"part{p}", name=f"part{p}")
                for p in range(NPAIR)
            ]

            # g for pair 0 ahead of the loop so the ACT queue never makes
            # g_{p+1} wait behind st_p
            gs = {}

            def emit_g(p):
                gs[p] = workp.tile([128, 1024], f16, tag="g", name=f"g{p}")
                utm = uts[p][:].rearrange("p q w -> p (q w)")
                nc.scalar.activation(gs[p][:], utm, Act.Copy, bias=1044.0,
                                     scale=50.0)

            emit_g(0)

            def emit_loss(p):
                # table chunk p complete: loss piece (3 pairs delayed so the
                # DVE queue never stalls on the scatter semaphore)
                e2 = lossp.tile([128, CHW], f32, tag="e2", name=f"e2_{p}")
                nc.vector.tensor_tensor(
                    out=e2[:], in0=refs_sb[p][:], in1=tbl[p][:],
                    op=Alu.subtract,
                )
                a = lossp.tile([128, CHW], f32, tag="a", name=f"a{p}")
                nc.scalar.activation(a[:], e2[:], Act.Abs)
                cm = lossp.tile([128, CHW], f32, tag="cm", name=f"cm{p}")
                nc.vector.tensor_scalar(
                    cm[:], a[:], 12.0, None, op0=Alu.is_lt
                )
                junk = lossp.tile([128, CHW], f32, tag="junk", name=f"junk{p}")
                nc.vector.scalar_tensor_tensor(
                    junk[:], a[:], 1.0, cm[:],
                    op0=Alu.mult, op1=Alu.mult,
                    accum_out=parts[p][:],
                )
                nc.scalar.dma_start(out[:, p:p + 1], parts[p][:])

            for p in range(NPAIR):
                if p + 3 < NPAIR:
                    fetch_ut(p + 3)
                if p + 2 < NPAIR:
                    fetch_refs(p + 2)
                if p + 1 < NPAIR:
                    emit_g(p + 1)

                ut = uts.pop(p)
                utm = ut[:].rearrange("p q w -> p (q w)")
                g = gs.pop(p)

                # mkx = (u < 0.0235)*4000 - cofs
                mk = workp.tile([128, 1024], f16, tag="mk")
                nc.vector.tensor_scalar(
                    mk[:], utm, 0.0235, MASK_SUB, op0=Alu.is_lt, op1=Alu.mult
                )
                mkx = workp.tile([128, 1024], f16, tag="mkx")
                nc.vector.tensor_tensor(
                    out=mkx[:], in0=mk[:], in1=cofs[:], op=Alu.subtract
                )
                # ix = g - mkx: bin+1044+cofs (valid) | very negative (masked)
                ix = workp.tile([128, 1024], f16, tag="ix")
                nc.vector.tensor_tensor(
                    out=ix[:], in0=g[:], in1=mkx[:], op=Alu.subtract
                )

                # transpose to partition=j; 8 blocks (k-parity, jt, h)
                ps = psump.tile([128, 1024], f16, tag="ps", space="PSUM")
                for kk in range(2):
                    o = kk * 512
                    nc.tensor.transpose(ps[:, o:o + 128],
                                        ix[:, o:o + 128], antid[:])
                    nc.tensor.transpose(ps[:, o + 128:o + 256],
                                        ix[:, o + 256:o + 384], ident[:])
                    nc.tensor.transpose(ps[:, o + 256:o + 384],
                                        ix[:, o + 128:o + 256], antid[:])
                    nc.tensor.transpose(ps[:, o + 384:o + 512],
                                        ix[:, o + 384:o + 512], ident[:])

                # idx = ps - 1045 in [0,400) valid, negative = skip
                st = workp.tile([128, 1024], i16, tag="st")
                nc.scalar.activation(st[:], ps[:], Act.Copy, bias=-1045.0,
                                     scale=1.0)

                nc.gpsimd.local_scatter(
                    tbl[p][:],
                    vee[:],
                    st[:],
                    channels=128,
                    num_elems=2 * KTBL,
                    num_idxs=1024,
                )
                if p >= 3:
                    emit_loss(p - 3)

            for p in range(NPAIR - 3, NPAIR):
                emit_loss(p)

    nc.compile()
    return nc


def _host_constants():
    # scatter data stream values per 256-block: [left: n+1 | right: n-128],
    # +OFF; garbage 25000 at the right-stream head (i==128, value 0 in the
    # reference -> must never produce a live table entry by itself)
    n = np.arange(256)
    blk = np.where(n < 128, n + 1, n - 128).astype(np.int64) + int(OFF)
    blk[128] = 25000
    vee = np.tile(blk, 4).astype(np.int16)
    vee = np.ascontiguousarray(np.broadcast_to(vee, (128, 1024)))

    # cofs[col]: block col offset {0,50,100,150} + 200*(k%2);
    # pre-transpose layout: col = (k%2)*512 + h*256 + j
    col = np.arange(1024)
    kk = col // 512
    h = (col % 512) // 256
    j = col % 256
    jt = (j // 128).astype(np.int64)
    cofs = (200 * kk + 100 * jt + 50 * h).astype(np.float32)
    cofs = np.ascontiguousarray(
        np.broadcast_to(cofs.astype(np.float16), (128, 1024))
    )

    ident = np.eye(128, dtype=np.float32).astype(np.float16)
    antid = np.ascontiguousarray(ident[::-1, :])
    return vee, cofs, ident, antid


def _prep_refs(left, right):
    """[CHUNKS*128, CHW] per core; table col (within core) =
    k*200 + block*50 + (bin-111), block in [jt0L, jt0R, jt1L, jt1R],
    channel = j mod 128; values pre-scaled: 60*ref + OFF."""
    lft = left[:, 0, :, 111:161]    # [B, W, 50]
    rgt = right[:, 0, :, 111:161]
    refs = np.empty((NCORES, KPC, 4, 128, NBIN), np.float32)
    for core in range(NCORES):
        for k in range(KPC):
            kg = core * KPC + k
            refs[core, k, 0] = lft[kg, 0:128, :]
            refs[core, k, 1] = rgt[kg, 0:128, :]
            refs[core, k, 2] = lft[kg, 128:256, :]
            refs[core, k, 3] = rgt[kg, 128:256, :]
    refs = refs * 60.0 + np.float32(OFF)
    # -> [core, pair, channel, (k_in_pair, block, bin)]
    refs = refs.reshape(NCORES, NPAIR, 2, 4, 128, NBIN)
    refs = refs.transpose(0, 1, 4, 2, 3, 5)
    return np.ascontiguousarray(
        refs.reshape(NCORES, NPAIR * 128, CHW).astype(np.float32)
    )


def make_in_maps(up, left, right):
    up = np.asarray(up, np.float32)
    left = np.asarray(left, np.float32)
    right = np.asarray(right, np.float32)
    vee, cofs, ident, antid = _host_constants()
    refs = _prep_refs(left, right)
    in_maps = []
    for c in range(NCORES):
        upc = np.ascontiguousarray(
            up[c * KPC:(c + 1) * KPC, 0].reshape(KPC * H, W)
        )
        in_maps.append({
            "up_in": upc,
            "refs_in": refs[c],
            "vee_in": vee,
            "cofs_in": cofs,
            "ident_in": ident,
            "antid_in": antid,
        })
    return in_maps


def get_nc():
    if "nc" not in _cache:
        _cache["nc"] = _build_bass()
    return _cache["nc"]


def reduce_results(results):
    total = 0.0
    for r in results:
        total += float(np.asarray(r["out"]).astype(np.float64).sum())
    return np.float32(total / (60.0 * B * W * W))


def kernel(up, left, right):
    nc = get_nc()
    in_maps = make_in_maps(up, left, right)
    res = run_bass_kernel_spmd(nc, in_maps, core_ids=list(range(NCORES)))
    return reduce_results(res.results)
